# revision 63
# baseline (speedup 1.0000x reference)
"""Trainium2 Bass kernel for GQA attention (B=2, S=2048, DIM=2048, H=16, KV=8,
HD=128) with RoPE + causal mask + output projection.

Sharding: 8-way tensor parallelism over heads. Core c computes q heads
{2c, 2c+1} and kv head c end-to-end (QKV projection, RoPE, causal attention),
contributes its transposed attention output to on-device AllGathers (Shared
output buffers), then computes the output-projection column slice
out[:, 256c:256(c+1)] from the gathered activations. The host only slices
inputs and concatenates outputs.

v5 pipeline notes:
- softmax denominator accumulated on the PE (ones-vector matmul per j block,
  PSUM-accumulated alongside the AV matmul) instead of a DVE add chain.
- projection PSUM windows evicted to bf16 SBUF immediately (ACT for the even
  halves, DVE PSUM-read-port copies for the odd halves); RoPE runs
  SBUF->SBUF in bf16 on the DVE off the PE critical path.
- finalize per chunk: reciprocal_approx_fast on the [1,512] denominator,
  ones-row broadcast matmul, ACT copy to bf16, one DVE multiply per head.
- wo block for chunk (b,t) is emitted two attention chunks after its
  AllGather fires, so the PE arrives after the collective completes; gather
  loads ride the gpsimd queue so they can never head-of-line-block the xt
  stream on the sync queue.
- AllGather outputs are Shared-address-space DRAM (faster RDH path).

Layout tricks:
- everything computed transposed (feature dim on SBUF partitions); only
  on-device transposes are 16 PE transposes per batch for v.
- RoPE interleaved pairs handled by permuting wq/wk columns on the host to
  [evens, odds] per head; q/k permuted consistently so dot products are
  unchanged; v / wo stay unpermuted.
- softmax in scoresT layout (keys on partitions): no max subtraction (scores
  are O(5)), causal mask as a -30 additive bias accumulated by the PE
  (identity matmul) on diagonal blocks only.
- matmuls bf16 (fp32 accumulate); 1/sqrt(HD) folded into wq.
"""

import sys

if "/opt/trn_rl_repo" not in sys.path:
    sys.path.insert(0, "/opt/trn_rl_repo")

import numpy as np
import ml_dtypes

B, S, DIM = 2, 2048, 2048
H, KV, HD = 16, 8, 128
NC = 8
NS = B * S            # 4096 flattened (b, s) rows
P = 128
MB = DIM // P         # 16 contraction blocks for the projections
BF = ml_dtypes.bfloat16

_cache: dict = {}


def _build(debug=False):
    import concourse.bass as bass
    import concourse.mybir as mybir
    import concourse.tile as tile
    from concourse import bacc
    from concourse.masks import make_identity

    dt = mybir.dt
    f32, bf16 = dt.float32, dt.bfloat16
    Exp = mybir.ActivationFunctionType.Exp

    nc = bacc.Bacc("TRN2", debug=False, target_bir_lowering=False, num_devices=NC)

    # x^T arrives pre-tiled as [m_block, window, 128, 512] so every
    # projection-stream DMA is one contiguous 128KB block
    xT_h = nc.dram_tensor("xT", (MB, 8, P, 512), bf16, kind="ExternalInput").ap()
    # weights arrive pre-tiled as [mi=128, mb*d] so their DMAs are contiguous
    wq_h = nc.dram_tensor("wq_c", (P, MB * 256), bf16, kind="ExternalInput").ap()
    wk_h = nc.dram_tensor("wk_c", (P, MB * HD), bf16, kind="ExternalInput").ap()
    wv_h = nc.dram_tensor("wv_c", (P, MB * HD), bf16, kind="ExternalInput").ap()
    wo_h = nc.dram_tensor("wo_c", (P, MB * 256), bf16, kind="ExternalInput").ap()
    cos_h = nc.dram_tensor("cosT", (64, NS), bf16, kind="ExternalInput").ap()
    sin_h = nc.dram_tensor("sinT", (64, NS), bf16, kind="ExternalInput").ap()
    # 0/1 upper-triangle for the within-block causal mask
    tri_h = nc.dram_tensor("tri", (P, P), bf16, kind="ExternalInput").ap()
    out_h = nc.dram_tensor("outT", (256, NS), bf16, kind="ExternalOutput").ap()

    with tile.TileContext(nc) as tc:
        with (
            tc.tile_pool(name="const", bufs=1) as const,
            tc.tile_pool(name="persist", bufs=1) as persist,
            tc.tile_pool(name="xs", bufs=16) as xs,
            tc.tile_pool(name="tmp", bufs=3) as tmp,
            tc.tile_pool(name="qk", bufs=2) as qk,
            tc.tile_pool(name="qk1", bufs=1) as qk1,
            tc.tile_pool(name="et", bufs=10) as et,
            tc.tile_pool(name="gp", bufs=8) as gp,
            tc.tile_pool(name="ot", bufs=3) as ot,
            tc.tile_pool(name="dram", bufs=1, space="DRAM") as dram,
        ):
            # ---- constants into SBUF ----
            # wq on the sync queue ahead of the xt stream; everything else on
            # the gpsimd queue.
            wq_sb = const.tile([P, MB, 256], bf16)
            nc.sync.dma_start(wq_sb[:], wq_h.rearrange("p (mb d) -> p mb d", mb=MB))
            wk_sb = const.tile([P, MB, HD], bf16)
            nc.gpsimd.dma_start(wk_sb[:], wk_h.rearrange("p (mb d) -> p mb d", mb=MB))
            wv_sb = const.tile([P, MB, HD], bf16)
            nc.gpsimd.dma_start(wv_sb[:], wv_h.rearrange("p (mb d) -> p mb d", mb=MB))
            cos_sb = const.tile([64, NS], bf16)
            nc.gpsimd.dma_start(cos_sb[:], cos_h)
            sin_sb = const.tile([64, NS], bf16)
            nc.gpsimd.dma_start(sin_sb[:], sin_h)
            wo_sb = const.tile([P, MB, 256], bf16)
            nc.gpsimd.dma_start(wo_sb[:], wo_h.rearrange("p (mb d) -> p mb d", mb=MB))
            tri_sb = const.tile([P, P], bf16)
            nc.gpsimd.dma_start(tri_sb[:], tri_h)
            ones_sb = const.tile([P, 1], bf16)
            nc.gpsimd.memset(ones_sb[:], 1.0)
            ident = const.tile([P, P], bf16)
            make_identity(nc, ident[:])

            # ---- per-batch persistent activations ----
            qrot = [persist.tile([P, 2, S], bf16, name=f"qrot{b}") for b in range(B)]
            krot = [persist.tile([P, S], bf16, name=f"krot{b}") for b in range(B)]
            vTt = [persist.tile([P, S], bf16, name=f"vTt{b}") for b in range(B)]
            vnat = [persist.tile([P, S // P, HD], bf16, name=f"vnat{b}")
                    for b in range(B)]
            oav = [persist.tile([P, 2, S], bf16, name=f"oav{b}") for b in range(B)]
            # exchange groups: chunks sharing one AllGather (batching
            # amortizes the ~9us fixed collective cost). Members are
            # (b, t, qlo, qhi) in processing order; the final chunk is split
            # into two 256-column halves so the tail collective is small.
            GROUPS = [
                [(0, 0, 0, 512)],
                [(0, 1, 0, 512)],
                [(0, 2, 0, 512)],
                [(0, 3, 0, 512), (1, 0, 0, 512)],
                [(1, 1, 0, 512), (1, 2, 0, 512)],
                [(1, 3, 0, 256)],
                [(1, 3, 256, 512)],
            ]
            slot_of = {}
            for gi, members in enumerate(GROUPS):
                for si, m in enumerate(members):
                    slot_of[m] = (gi, si)
            ag_in = [dram.tile([len(m) * 256, m[0][3] - m[0][2]], bf16,
                               name=f"agi{gi}")
                     for gi, m in enumerate(GROUPS)]
            ag_out = [dram.tile([NC * len(m) * 256, m[0][3] - m[0][2]], bf16,
                                name=f"ago{gi}", addr_space="Shared")
                      for gi, m in enumerate(GROUPS)]

            # tiny warmup AllGather fired during the projection phase so the
            # first real collective doesn't pay the slow-start cost
            warm_in = dram.tile([1, 512], bf16, name="warm_in")
            warm_out = dram.tile([NC, 512], bf16, name="warm_out",
                                 addr_space="Shared")
            nc.gpsimd.dma_start(warm_in[:], cos_h[0:1, 0:512])
            nc.gpsimd.collective_compute(
                "AllGather",
                mybir.AluOpType.bypass,
                replica_groups=[list(range(NC))],
                ins=[warm_in.opt()],
                outs=[warm_out.opt()],
            )

            def rope_unit(xe, xo, cos_c, sin_c, out_even, out_odd):
                # xe/xo are bf16 base-0 SBUF copies of the window's two halves
                t1 = tmp.tile([64, 512], bf16, tag="r1", name="r1")
                t2 = tmp.tile([64, 512], bf16, tag="r2", name="r2")
                nc.vector.tensor_mul(t1[:], xe, cos_c)
                nc.vector.tensor_mul(t2[:], xo, sin_c)
                nc.vector.tensor_sub(out_even, t1[:], t2[:])
                t3 = tmp.tile([64, 512], bf16, tag="r1", name="r3")
                t4 = tmp.tile([64, 512], bf16, tag="r2", name="r4")
                nc.vector.tensor_mul(t3[:], xe, sin_c)
                nc.vector.tensor_mul(t4[:], xo, cos_c)
                nc.vector.tensor_add(out_odd, t3[:], t4[:])

            def wo_exchange(m):
                """Stage member m=(b,t,qlo,qhi)'s attention output into its
                group slot; fire the group's AllGather once its last member
                is staged."""
                b, t, qlo, qhi = m
                gi, si = slot_of[m]
                il = slice(t * 512 + qlo, t * 512 + qhi)
                for h in range(2):
                    nc.gpsimd.dma_start(
                        ag_in[gi][si * 256 + h * P: si * 256 + (h + 1) * P, :],
                        oav[b][:, h, il])
                if si == len(GROUPS[gi]) - 1:
                    nc.gpsimd.collective_compute(
                        "AllGather",
                        mybir.AluOpType.bypass,
                        replica_groups=[list(range(NC))],
                        ins=[ag_in[gi].opt()],
                        outs=[ag_out[gi].opt()],
                    )

            def wo_block(m, pool):
                """Output-projection column slice for member m: gathered rows
                arrive as paired loads alternating between the gpsimd and
                sync queues, then 32 PE matmuls + bf16 store."""
                b, t, qlo, qhi = m
                w = qhi - qlo
                gi, si = slot_of[m]
                rs = len(GROUPS[gi]) * 256   # gathered rows per core
                pw = [pool.tile([P, 512], f32, tag="ps", name=f"pw{n}")
                      for n in range(2)]
                for rp in range(MB // 2):
                    g2 = gp.tile([P, 2, 512], bf16, tag="g", name="g")
                    base = rp * rs + si * 256
                    src = ag_out[gi][base:base + 256, :].rearrange(
                        "(two p) q -> p two q", two=2)
                    if rp % 2 == 0:
                        nc.gpsimd.dma_start(g2[:, :, 0:w], src)
                    else:
                        nc.sync.dma_start(g2[:, :, 0:w], src)
                    for half in range(2):
                        r = rp * 2 + half
                        for n in range(2):
                            nc.tensor.matmul(
                                pw[n][:, 0:w],
                                wo_sb[:, r, n * 128:(n + 1) * 128],
                                g2[:, half, 0:w],
                                start=(r == 0), stop=(r == MB - 1),
                            )
                for n in range(2):
                    o = ot.tile([P, 512], bf16, tag="o", name="o")
                    nc.scalar.copy(o[:, 0:w], pw[n][:, 0:w])
                    nc.sync.dma_start(
                        out_h[n * P:(n + 1) * P,
                              b * S + t * 512 + qlo: b * S + t * 512 + qhi],
                        o[:, 0:w],
                    )

            # wo blocks become runnable when their group's AllGather fires
            wo_ready = []
            # b1 rope units deferred into the b0 attention phase
            rope_deferred = []

            def attn_chunk(m, psS, psV, psD):
                """Causal attention for query cols [qlo,qhi) of chunk (b,t)
                in scoresT layout, incl. softmax finalize and exchange."""
                b, t, qlo, qhi = m
                w = qhi - qlo
                il = slice(t * 512 + qlo, t * 512 + qhi)
                pav = [psV.tile([P, 512], f32, tag=f"pav{h}",
                                name=f"pav{h}") for h in range(2)]
                pden = [psD.tile([1, 512], f32, tag=f"pden{h}",
                                 name=f"pden{h}") for h in range(2)]
                nj = (t * 512 + qhi - 1) // P + 1
                # descending j: masked diagonal blocks run first so the
                # drain at the end only waits on plain exps
                order = list(range(nj - 1, -1, -1))

                def av_den(e, idx, j, h):
                    nc.tensor.matmul(
                        pden[h][:, 0:w], ones_sb[:], e[:, 0:w],
                        start=(idx == 0), stop=(idx == nj - 1),
                    )
                    nc.tensor.matmul(
                        pav[h][:, 0:w], vnat[b][:, j, :], e[:, 0:w],
                        start=(idx == 0), stop=(idx == nj - 1),
                    )

                pipe = []
                for idx, j in enumerate(order):
                    for h in range(2):
                        # column offset of key block j's diagonal in this
                        # query window
                        moff = j * P - (t * 512 + qlo)
                        ps = psS.tile([P, 512], f32, tag="ps", name="ps")
                        nc.tensor.matmul(
                            ps[:, 0:w], krot[b][:, j * P:(j + 1) * P],
                            qrot[b][:, h, il], start=True, stop=True,
                        )
                        e = et.tile([P, 512], bf16, tag="e", name="e")
                        if moff > 0:
                            # diagonal block: exp only the live columns;
                            # DVE zeroes the masked ones
                            nc.scalar.activation(
                                e[:, moff:w], ps[:, moff:w], Exp)
                            nc.vector.memset(e[:, 0:moff], 0.0)
                        else:
                            nc.scalar.activation(e[:, 0:w], ps[:, 0:w], Exp)
                        if moff >= 0:
                            # 0/1 triangle multiply on the diagonal
                            nc.vector.tensor_mul(
                                e[:, moff:moff + P],
                                e[:, moff:moff + P], tri_sb[:])
                        pipe.append((e, idx, j, h))
                    while len(pipe) > 6:
                        av_den(*pipe.pop(0))
                for item in pipe:
                    av_den(*item)

                # finalize: rcp(den), gpsimd partition broadcast, one DVE
                # multiply per head
                for h in range(2):
                    rcp = tmp.tile([1, 512], f32, tag="rcp", name="rcp")
                    nc.vector.reciprocal_approx_fast(rcp[:, 0:w],
                                                     pden[h][:, 0:w])
                    rcp_bf = tmp.tile([1, 512], bf16, tag="rcpc", name="rcpc")
                    nc.vector.tensor_copy(rcp_bf[:, 0:w], rcp[:, 0:w])
                    rcp_b = tmp.tile([P, 512], bf16, tag="rcpb", name="rcpb")
                    nc.gpsimd.partition_broadcast(rcp_b[:, 0:w],
                                                  rcp_bf[:, 0:w])
                    nc.vector.tensor_mul(oav[b][:, h, il],
                                         pav[h][:, 0:w], rcp_b[:, 0:w])

                wo_exchange(m)
                gi, si = slot_of[m]
                if si == len(GROUPS[gi]) - 1:
                    wo_ready.extend(GROUPS[gi])

            # ---- pass 1: projections + transposes for both batches, with
            # b0's first attention chunk squeezed between them so the
            # AllGather stream starts ~100us earlier ----
            for b in range(B):
                with tc.tile_pool(name=f"psA{b}", bufs=2, space="PSUM") as psA:
                    for sp in range(4):          # 512-col windows within batch
                        gw = slice(b * S + sp * 512, b * S + (sp + 1) * 512)
                        lw = slice(sp * 512, (sp + 1) * 512)
                        pq = [psA.tile([P, 512], f32, tag=f"pq{h}", name=f"pq{h}")
                              for h in range(2)]
                        pk = psA.tile([P, 512], f32, tag="pk", name="pk")
                        pv = psA.tile([P, 512], f32, tag="pv", name="pv")
                        for m in range(MB):
                            xt = xs.tile([P, 512], bf16, tag="xt", name="xt")
                            nc.sync.dma_start(xt[:], xT_h[m, b * 4 + sp])
                            for acc, lhsT in (
                                (pq[0], wq_sb[:, m, 0:128]),
                                (pq[1], wq_sb[:, m, 128:256]),
                                (pk, wk_sb[:, m, :]),
                                (pv, wv_sb[:, m, :]),
                            ):
                                nc.tensor.matmul(
                                    acc[:], lhsT, xt[:],
                                    start=(m == 0), stop=(m == MB - 1),
                                )
                        # evict the PSUM banks to bf16 SBUF right away: even
                        # halves via ACT, odd halves via the DVE PSUM read
                        # port (which supports the base-64 partition offset).
                        # b1's last two windows get dedicated tiles because
                        # their RoPE is deferred into the b0 attention phase
                        # (keeps the DVE queue clear for b0's finalize chains)
                        defer = b == 1 and sp >= 2
                        halves = []
                        for u, src in enumerate((pq[0], pq[1], pk)):
                            if defer:
                                xe = qk1.tile([64, 512], bf16,
                                              tag=f"dxe{sp}{u}", name=f"xe{u}")
                                xo = qk1.tile([64, 512], bf16,
                                              tag=f"dxo{sp}{u}", name=f"xo{u}")
                            else:
                                xe = qk.tile([64, 512], bf16, tag=f"xe{u}",
                                             name=f"xe{u}")
                                xo = qk.tile([64, 512], bf16, tag=f"xo{u}",
                                             name=f"xo{u}")
                            nc.scalar.copy(xe[:], src[0:64, :])
                            nc.vector.tensor_copy(xo[:], src[64:128, :])
                            halves.append((xe, xo))
                        nc.scalar.copy(vTt[b][:, lw], pv[:])
                        # RoPE runs SBUF->SBUF on the DVE, off the PE path
                        cos_c, sin_c = cos_sb[:, gw], sin_sb[:, gw]
                        if defer:
                            rope_deferred.append((halves, cos_c, sin_c, lw))
                        else:
                            for h in range(2):
                                rope_unit(halves[h][0][:], halves[h][1][:],
                                          cos_c, sin_c,
                                          qrot[b][0:64, h, lw],
                                          qrot[b][64:128, h, lw])
                            rope_unit(halves[2][0][:], halves[2][1][:],
                                      cos_c, sin_c,
                                      krot[b][0:64, lw], krot[b][64:128, lw])

                # ---- v natural layout via PE transposes ----
                with tc.tile_pool(name=f"psT{b}", bufs=2, space="PSUM") as psT:
                    for blk in range(S // P):
                        pt = psT.tile([P, P], bf16, tag="pt", name="pt")
                        nc.tensor.transpose(
                            pt[:], vTt[b][:, blk * P:(blk + 1) * P], ident[:])
                        nc.scalar.copy(vnat[b][:, blk, :], pt[:])

                if b == 0:
                    # early attention chunk (0,0): fires the first real
                    # AllGather while b1's projections still run
                    with (
                        tc.tile_pool(name="psSe", bufs=3, space="PSUM") as psSe,
                        tc.tile_pool(name="psVe", bufs=1, space="PSUM") as psVe,
                        tc.tile_pool(name="psDe", bufs=1, space="PSUM") as psDe,
                    ):
                        attn_chunk((0, 0, 0, 512), psSe, psVe, psDe)

            # ---- pass 2: causal attention in scoresT layout ----
            for b in range(B):
                with (
                    tc.tile_pool(name=f"psS{b}", bufs=4, space="PSUM") as psS,
                    tc.tile_pool(name=f"psV{b}", bufs=1, space="PSUM") as psV,
                    tc.tile_pool(name=f"psD{b}", bufs=1, space="PSUM") as psD,
                ):
                    if b == 0:
                        members = [(0, 1, 0, 512), (0, 2, 0, 512),
                                   (0, 3, 0, 512)]
                    else:
                        members = [(1, 0, 0, 512), (1, 1, 0, 512),
                                   (1, 2, 0, 512), (1, 3, 0, 256),
                                   (1, 3, 256, 512)]
                    for m in members:             # query chunks
                        attn_chunk(m, psS, psV, psD)
                        # one deferred b1 rope window per b0 chunk, emitted
                        # right after the finalize chain on the DVE queue
                        if b == 0 and rope_deferred:
                            halves_d, cos_d, sin_d, lw_d = rope_deferred.pop(0)
                            for h in range(2):
                                rope_unit(halves_d[h][0][:], halves_d[h][1][:],
                                          cos_d, sin_d,
                                          qrot[1][0:64, h, lw_d],
                                          qrot[1][64:128, h, lw_d])
                            rope_unit(halves_d[2][0][:], halves_d[2][1][:],
                                      cos_d, sin_d,
                                      krot[1][0:64, lw_d], krot[1][64:128, lw_d])
                    # all wo blocks drain after the last exchange fires: the
                    # ~56us of PE work here fully covers the CC stream tail
                    if b == B - 1:
                        while wo_ready:
                            wo_block(wo_ready.pop(0), psS)

    nc.compile()
    return nc


def _prep_inputs(x, freqs_cos, freqs_sin, wq, wk, wv, wo):
    x = np.asarray(x, np.float32).reshape(NS, DIM)
    xT = np.ascontiguousarray(
        x.T.reshape(MB, P, 8, 512).transpose(0, 2, 1, 3)).astype(BF)
    cos = np.asarray(freqs_cos, np.float32)
    sin = np.asarray(freqs_sin, np.float32)
    cosT = np.ascontiguousarray(np.tile(cos, (B, 1)).T).astype(BF)
    sinT = np.ascontiguousarray(np.tile(sin, (B, 1)).T).astype(BF)

    perm = np.r_[np.arange(0, HD, 2), np.arange(1, HD, 2)]
    scale = np.float32(1.0 / np.sqrt(HD))
    wq = np.asarray(wq, np.float32) * scale
    wk = np.asarray(wk, np.float32)
    wv = np.asarray(wv, np.float32)
    wo = np.asarray(wo, np.float32)

    tri = np.triu(np.ones((P, P), np.float32)).astype(BF)

    def tile_w(w):
        # (2048, d) -> (128, 16*d): row mi holds [mb, d] contiguously
        d = w.shape[1]
        return np.ascontiguousarray(
            w.reshape(MB, P, d).transpose(1, 0, 2).reshape(P, MB * d)).astype(BF)

    in_maps = []
    for c in range(NC):
        wq_c = wq[:, c * 256:(c + 1) * 256]
        wq_cp = np.concatenate([wq_c[:, h * HD + perm] for h in range(2)], axis=1)
        in_maps.append({
            "xT": xT,
            "wq_c": tile_w(wq_cp),
            "wk_c": tile_w(wk[:, c * HD:(c + 1) * HD][:, perm]),
            "wv_c": tile_w(wv[:, c * HD:(c + 1) * HD]),
            "wo_c": tile_w(wo[:, c * 256:(c + 1) * 256]),
            "cosT": cosT,
            "sinT": sinT,
            "tri": tri,
        })
    return in_maps


def _run(inputs, trace=False, **kw):
    from concourse.bass_utils import run_bass_kernel_spmd

    if "nc" not in _cache:
        _cache["nc"] = _build()
    nc = _cache["nc"]
    in_maps = _prep_inputs(**inputs)
    res = run_bass_kernel_spmd(
        nc, in_maps, core_ids=list(range(NC)), trace=trace, **kw
    )
    out = np.empty((NS, DIM), np.float32)
    for c in range(NC):
        out[:, c * 256:(c + 1) * 256] = np.asarray(
            res.results[c]["outT"], dtype=np.float32).T
    return out.reshape(B, S, DIM), res


def kernel(**inputs) -> np.ndarray:
    out, _ = _run(inputs, trace=False)
    return out


# revision 65
# speedup vs baseline: 1.0928x; 1.0928x over previous
"""Trainium2 Bass kernel for GQA attention (B=2, S=2048, DIM=2048, H=16, KV=8,
HD=128) with RoPE + causal mask + output projection.

Sharding: 8-way tensor parallelism over heads. Core c computes q heads
{2c, 2c+1} and kv head c end-to-end (QKV projection, RoPE, causal attention),
contributes its transposed attention output to on-device AllGathers (Shared
output buffers), then computes the output-projection column slice
out[:, 256c:256(c+1)] from the gathered activations. The host only slices
inputs and concatenates outputs.

v5 pipeline notes:
- softmax denominator accumulated on the PE (ones-vector matmul per j block,
  PSUM-accumulated alongside the AV matmul) instead of a DVE add chain.
- projection PSUM windows evicted to bf16 SBUF immediately (ACT for the even
  halves, DVE PSUM-read-port copies for the odd halves); RoPE runs
  SBUF->SBUF in bf16 on the DVE off the PE critical path.
- finalize per chunk: reciprocal_approx_fast on the [1,512] denominator,
  ones-row broadcast matmul, ACT copy to bf16, one DVE multiply per head.
- wo block for chunk (b,t) is emitted two attention chunks after its
  AllGather fires, so the PE arrives after the collective completes; gather
  loads ride the gpsimd queue so they can never head-of-line-block the xt
  stream on the sync queue.
- AllGather outputs are Shared-address-space DRAM (faster RDH path).

Layout tricks:
- everything computed transposed (feature dim on SBUF partitions); only
  on-device transposes are 16 PE transposes per batch for v.
- RoPE interleaved pairs handled by permuting wq/wk columns on the host to
  [evens, odds] per head; q/k permuted consistently so dot products are
  unchanged; v / wo stay unpermuted.
- softmax in scoresT layout (keys on partitions): no max subtraction (scores
  are O(5)), causal mask as a -30 additive bias accumulated by the PE
  (identity matmul) on diagonal blocks only.
- matmuls bf16 (fp32 accumulate); 1/sqrt(HD) folded into wq.
"""

import sys

if "/opt/trn_rl_repo" not in sys.path:
    sys.path.insert(0, "/opt/trn_rl_repo")

import numpy as np
import ml_dtypes

B, S, DIM = 2, 2048, 2048
H, KV, HD = 16, 8, 128
NC = 8
NS = B * S            # 4096 flattened (b, s) rows
P = 128
MB = DIM // P         # 16 contraction blocks for the projections
BF = ml_dtypes.bfloat16

_cache: dict = {}


def _build(debug=False):
    import concourse.bass as bass
    import concourse.mybir as mybir
    import concourse.tile as tile
    from concourse import bacc
    from concourse.masks import make_identity

    dt = mybir.dt
    f32, bf16 = dt.float32, dt.bfloat16
    Exp = mybir.ActivationFunctionType.Exp

    nc = bacc.Bacc("TRN2", debug=False, target_bir_lowering=False, num_devices=NC)

    # x^T arrives pre-tiled as [m_block, window, 128, 512] so every
    # projection-stream DMA is one contiguous 128KB block
    xT_h = nc.dram_tensor("xT", (MB, 8, P, 512), bf16, kind="ExternalInput").ap()
    # weights arrive pre-tiled as [mi=128, mb*d] so their DMAs are contiguous
    wq_h = nc.dram_tensor("wq_c", (P, MB * 256), bf16, kind="ExternalInput").ap()
    wk_h = nc.dram_tensor("wk_c", (P, MB * HD), bf16, kind="ExternalInput").ap()
    wv_h = nc.dram_tensor("wv_c", (P, MB * HD), bf16, kind="ExternalInput").ap()
    wo_h = nc.dram_tensor("wo_c", (P, MB * 256), bf16, kind="ExternalInput").ap()
    cos_h = nc.dram_tensor("cosT", (64, NS), bf16, kind="ExternalInput").ap()
    sin_h = nc.dram_tensor("sinT", (64, NS), bf16, kind="ExternalInput").ap()
    # 0/1 upper-triangle for the within-block causal mask
    tri_h = nc.dram_tensor("tri", (P, P), bf16, kind="ExternalInput").ap()
    out_h = nc.dram_tensor("outT", (256, NS), bf16, kind="ExternalOutput").ap()

    with tile.TileContext(nc) as tc:
        with (
            tc.tile_pool(name="const", bufs=1) as const,
            tc.tile_pool(name="persist", bufs=1) as persist,
            tc.tile_pool(name="xs", bufs=16) as xs,
            tc.tile_pool(name="tmp", bufs=3) as tmp,
            tc.tile_pool(name="qk", bufs=2) as qk,
            tc.tile_pool(name="qk1", bufs=1) as qk1,
            tc.tile_pool(name="et", bufs=10) as et,
            tc.tile_pool(name="gp", bufs=8) as gp,
            tc.tile_pool(name="ot", bufs=3) as ot,
            tc.tile_pool(name="dram", bufs=1, space="DRAM") as dram,
        ):
            # ---- constants into SBUF ----
            # wq on the sync queue ahead of the xt stream; everything else on
            # the gpsimd queue.
            wq_sb = const.tile([P, MB, 256], bf16)
            nc.sync.dma_start(wq_sb[:], wq_h.rearrange("p (mb d) -> p mb d", mb=MB))
            wk_sb = const.tile([P, MB, HD], bf16)
            nc.gpsimd.dma_start(wk_sb[:], wk_h.rearrange("p (mb d) -> p mb d", mb=MB))
            wv_sb = const.tile([P, MB, HD], bf16)
            nc.gpsimd.dma_start(wv_sb[:], wv_h.rearrange("p (mb d) -> p mb d", mb=MB))
            cos_sb = const.tile([64, NS], bf16)
            nc.gpsimd.dma_start(cos_sb[:], cos_h)
            sin_sb = const.tile([64, NS], bf16)
            nc.gpsimd.dma_start(sin_sb[:], sin_h)
            wo_sb = const.tile([P, MB, 256], bf16)
            nc.gpsimd.dma_start(wo_sb[:], wo_h.rearrange("p (mb d) -> p mb d", mb=MB))
            tri_sb = const.tile([P, P], bf16)
            nc.gpsimd.dma_start(tri_sb[:], tri_h)
            ones_sb = const.tile([P, 1], bf16)
            nc.gpsimd.memset(ones_sb[:], 1.0)
            ident = const.tile([P, P], bf16)
            make_identity(nc, ident[:])

            # ---- per-batch persistent activations ----
            qrot = [persist.tile([P, 2, S], bf16, name=f"qrot{b}") for b in range(B)]
            krot = [persist.tile([P, S], bf16, name=f"krot{b}") for b in range(B)]
            vTt = [persist.tile([P, S], bf16, name=f"vTt{b}") for b in range(B)]
            vnat = [persist.tile([P, S // P, HD], bf16, name=f"vnat{b}")
                    for b in range(B)]
            oav = [persist.tile([P, 2, S], bf16, name=f"oav{b}") for b in range(B)]
            # exchange groups: chunks sharing one AllGather (batching
            # amortizes the ~9us fixed collective cost). Members are
            # (b, t, qlo, qhi) in processing order; the final chunk is split
            # into two 256-column halves so the tail collective is small.
            GROUPS = [
                [(0, 0, 0, 512)],
                [(0, 1, 0, 512), (0, 2, 0, 512)],
                [(0, 3, 0, 512), (1, 0, 0, 512)],
                [(1, 1, 0, 512), (1, 2, 0, 512)],
                [(1, 3, 0, 512)],
            ]
            slot_of = {}
            for gi, members in enumerate(GROUPS):
                for si, m in enumerate(members):
                    slot_of[m] = (gi, si)
            ag_in = [dram.tile([len(m) * 256, m[0][3] - m[0][2]], bf16,
                               name=f"agi{gi}")
                     for gi, m in enumerate(GROUPS)]
            ag_out = [dram.tile([NC * len(m) * 256, m[0][3] - m[0][2]], bf16,
                                name=f"ago{gi}", addr_space="Shared")
                      for gi, m in enumerate(GROUPS)]

            # tiny warmup AllGather fired during the projection phase so the
            # first real collective doesn't pay the slow-start cost
            warm_in = dram.tile([1, 512], bf16, name="warm_in")
            warm_out = dram.tile([NC, 512], bf16, name="warm_out",
                                 addr_space="Shared")
            nc.gpsimd.dma_start(warm_in[:], cos_h[0:1, 0:512])
            nc.gpsimd.collective_compute(
                "AllGather",
                mybir.AluOpType.bypass,
                replica_groups=[list(range(NC))],
                ins=[warm_in.opt()],
                outs=[warm_out.opt()],
            )

            def rope_unit(xe, xo, cos_c, sin_c, out_even, out_odd):
                # xe/xo are bf16 base-0 SBUF copies of the window's two halves
                t1 = tmp.tile([64, 512], bf16, tag="r1", name="r1")
                t2 = tmp.tile([64, 512], bf16, tag="r2", name="r2")
                nc.vector.tensor_mul(t1[:], xe, cos_c)
                nc.vector.tensor_mul(t2[:], xo, sin_c)
                nc.vector.tensor_sub(out_even, t1[:], t2[:])
                t3 = tmp.tile([64, 512], bf16, tag="r1", name="r3")
                t4 = tmp.tile([64, 512], bf16, tag="r2", name="r4")
                nc.vector.tensor_mul(t3[:], xe, sin_c)
                nc.vector.tensor_mul(t4[:], xo, cos_c)
                nc.vector.tensor_add(out_odd, t3[:], t4[:])

            def wo_exchange(m):
                """Stage member m=(b,t,qlo,qhi)'s attention output into its
                group slot; fire the group's AllGather once its last member
                is staged."""
                b, t, qlo, qhi = m
                gi, si = slot_of[m]
                il = slice(t * 512 + qlo, t * 512 + qhi)
                for h in range(2):
                    nc.gpsimd.dma_start(
                        ag_in[gi][si * 256 + h * P: si * 256 + (h + 1) * P, :],
                        oav[b][:, h, il])
                if si == len(GROUPS[gi]) - 1:
                    nc.gpsimd.collective_compute(
                        "AllGather",
                        mybir.AluOpType.bypass,
                        replica_groups=[list(range(NC))],
                        ins=[ag_in[gi].opt()],
                        outs=[ag_out[gi].opt()],
                    )

            def wo_block(m, pool):
                """Output-projection column slice for member m: gathered rows
                arrive as paired loads alternating between the gpsimd and
                sync queues, then 32 PE matmuls + bf16 store."""
                b, t, qlo, qhi = m
                w = qhi - qlo
                gi, si = slot_of[m]
                rs = len(GROUPS[gi]) * 256   # gathered rows per core
                pw = [pool.tile([P, 512], f32, tag="ps", name=f"pw{n}")
                      for n in range(2)]
                for rp in range(MB // 2):
                    g2 = gp.tile([P, 2, 512], bf16, tag="g", name="g")
                    base = rp * rs + si * 256
                    src = ag_out[gi][base:base + 256, :].rearrange(
                        "(two p) q -> p two q", two=2)
                    if rp % 2 == 0:
                        nc.gpsimd.dma_start(g2[:, :, 0:w], src)
                    else:
                        nc.sync.dma_start(g2[:, :, 0:w], src)
                    for half in range(2):
                        r = rp * 2 + half
                        for n in range(2):
                            nc.tensor.matmul(
                                pw[n][:, 0:w],
                                wo_sb[:, r, n * 128:(n + 1) * 128],
                                g2[:, half, 0:w],
                                start=(r == 0), stop=(r == MB - 1),
                            )
                for n in range(2):
                    o = ot.tile([P, 512], bf16, tag="o", name="o")
                    nc.scalar.copy(o[:, 0:w], pw[n][:, 0:w])
                    nc.sync.dma_start(
                        out_h[n * P:(n + 1) * P,
                              b * S + t * 512 + qlo: b * S + t * 512 + qhi],
                        o[:, 0:w],
                    )

            # wo blocks become runnable when their group's AllGather fires
            wo_ready = []
            # b1 rope units deferred into the b0 attention phase
            rope_deferred = []

            def attn_chunk(m, psS, psV, psD):
                """Causal attention for query cols [qlo,qhi) of chunk (b,t)
                in scoresT layout, incl. softmax finalize and exchange."""
                b, t, qlo, qhi = m
                w = qhi - qlo
                il = slice(t * 512 + qlo, t * 512 + qhi)
                pav = [psV.tile([P, 512], f32, tag=f"pav{h}",
                                name=f"pav{h}") for h in range(2)]
                pden = [psD.tile([1, 512], f32, tag=f"pden{h}",
                                 name=f"pden{h}") for h in range(2)]
                nj = (t * 512 + qhi - 1) // P + 1
                # descending j: masked diagonal blocks run first so the
                # drain at the end only waits on plain exps
                order = list(range(nj - 1, -1, -1))

                def av_den(e, idx, j, h):
                    nc.tensor.matmul(
                        pden[h][:, 0:w], ones_sb[:], e[:, 0:w],
                        start=(idx == 0), stop=(idx == nj - 1),
                    )
                    nc.tensor.matmul(
                        pav[h][:, 0:w], vnat[b][:, j, :], e[:, 0:w],
                        start=(idx == 0), stop=(idx == nj - 1),
                    )

                pipe = []
                for idx, j in enumerate(order):
                    for h in range(2):
                        # column offset of key block j's diagonal in this
                        # query window
                        moff = j * P - (t * 512 + qlo)
                        ps = psS.tile([P, 512], f32, tag="ps", name="ps")
                        nc.tensor.matmul(
                            ps[:, 0:w], krot[b][:, j * P:(j + 1) * P],
                            qrot[b][:, h, il], start=True, stop=True,
                        )
                        e = et.tile([P, 512], bf16, tag="e", name="e")
                        if moff > 0:
                            # diagonal block: exp only the live columns;
                            # DVE zeroes the masked ones
                            nc.scalar.activation(
                                e[:, moff:w], ps[:, moff:w], Exp)
                            nc.vector.memset(e[:, 0:moff], 0.0)
                        else:
                            nc.scalar.activation(e[:, 0:w], ps[:, 0:w], Exp)
                        if moff >= 0:
                            # 0/1 triangle multiply on the diagonal
                            nc.vector.tensor_mul(
                                e[:, moff:moff + P],
                                e[:, moff:moff + P], tri_sb[:])
                        pipe.append((e, idx, j, h))
                    while len(pipe) > 6:
                        av_den(*pipe.pop(0))
                for item in pipe:
                    av_den(*item)

                # finalize: rcp(den), gpsimd partition broadcast, one DVE
                # multiply per head
                for h in range(2):
                    rcp = tmp.tile([1, 512], f32, tag="rcp", name="rcp")
                    nc.vector.reciprocal_approx_fast(rcp[:, 0:w],
                                                     pden[h][:, 0:w])
                    rcp_bf = tmp.tile([1, 512], bf16, tag="rcpc", name="rcpc")
                    nc.vector.tensor_copy(rcp_bf[:, 0:w], rcp[:, 0:w])
                    rcp_b = tmp.tile([P, 512], bf16, tag="rcpb", name="rcpb")
                    nc.gpsimd.partition_broadcast(rcp_b[:, 0:w],
                                                  rcp_bf[:, 0:w])
                    nc.vector.tensor_mul(oav[b][:, h, il],
                                         pav[h][:, 0:w], rcp_b[:, 0:w])

                wo_exchange(m)
                gi, si = slot_of[m]
                if si == len(GROUPS[gi]) - 1:
                    wo_ready.extend(GROUPS[gi])

            # ---- pass 1: projections + transposes for both batches, with
            # b0's first attention chunk squeezed between them so the
            # AllGather stream starts ~100us earlier ----
            for b in range(B):
                with tc.tile_pool(name=f"psA{b}", bufs=2, space="PSUM") as psA:
                    for sp in range(4):          # 512-col windows within batch
                        gw = slice(b * S + sp * 512, b * S + (sp + 1) * 512)
                        lw = slice(sp * 512, (sp + 1) * 512)
                        pq = [psA.tile([P, 512], f32, tag=f"pq{h}", name=f"pq{h}")
                              for h in range(2)]
                        pk = psA.tile([P, 512], f32, tag="pk", name="pk")
                        pv = psA.tile([P, 512], f32, tag="pv", name="pv")
                        for m in range(MB):
                            xt = xs.tile([P, 512], bf16, tag="xt", name="xt")
                            nc.sync.dma_start(xt[:], xT_h[m, b * 4 + sp])
                            for acc, lhsT in (
                                (pq[0], wq_sb[:, m, 0:128]),
                                (pq[1], wq_sb[:, m, 128:256]),
                                (pk, wk_sb[:, m, :]),
                                (pv, wv_sb[:, m, :]),
                            ):
                                nc.tensor.matmul(
                                    acc[:], lhsT, xt[:],
                                    start=(m == 0), stop=(m == MB - 1),
                                )
                        # evict the PSUM banks to bf16 SBUF right away: even
                        # halves via ACT, odd halves via the DVE PSUM read
                        # port (which supports the base-64 partition offset).
                        # b1's last two windows get dedicated tiles because
                        # their RoPE is deferred into the b0 attention phase
                        # (keeps the DVE queue clear for b0's finalize chains)
                        defer = b == 1 and sp >= 2
                        halves = []
                        for u, src in enumerate((pq[0], pq[1], pk)):
                            if defer:
                                xe = qk1.tile([64, 512], bf16,
                                              tag=f"dxe{sp}{u}", name=f"xe{u}")
                                xo = qk1.tile([64, 512], bf16,
                                              tag=f"dxo{sp}{u}", name=f"xo{u}")
                            else:
                                xe = qk.tile([64, 512], bf16, tag=f"xe{u}",
                                             name=f"xe{u}")
                                xo = qk.tile([64, 512], bf16, tag=f"xo{u}",
                                             name=f"xo{u}")
                            nc.scalar.copy(xe[:], src[0:64, :])
                            nc.vector.tensor_copy(xo[:], src[64:128, :])
                            halves.append((xe, xo))
                        nc.scalar.copy(vTt[b][:, lw], pv[:])
                        # RoPE runs SBUF->SBUF on the DVE, off the PE path
                        cos_c, sin_c = cos_sb[:, gw], sin_sb[:, gw]
                        if defer:
                            rope_deferred.append((halves, cos_c, sin_c, lw))
                        else:
                            for h in range(2):
                                rope_unit(halves[h][0][:], halves[h][1][:],
                                          cos_c, sin_c,
                                          qrot[b][0:64, h, lw],
                                          qrot[b][64:128, h, lw])
                            rope_unit(halves[2][0][:], halves[2][1][:],
                                      cos_c, sin_c,
                                      krot[b][0:64, lw], krot[b][64:128, lw])

                # ---- v natural layout via PE transposes ----
                with tc.tile_pool(name=f"psT{b}", bufs=2, space="PSUM") as psT:
                    for blk in range(S // P):
                        pt = psT.tile([P, P], bf16, tag="pt", name="pt")
                        nc.tensor.transpose(
                            pt[:], vTt[b][:, blk * P:(blk + 1) * P], ident[:])
                        nc.scalar.copy(vnat[b][:, blk, :], pt[:])

                if b == 0:
                    # early attention chunk (0,0): fires the first real
                    # AllGather while b1's projections still run
                    with (
                        tc.tile_pool(name="psSe", bufs=3, space="PSUM") as psSe,
                        tc.tile_pool(name="psVe", bufs=1, space="PSUM") as psVe,
                        tc.tile_pool(name="psDe", bufs=1, space="PSUM") as psDe,
                    ):
                        attn_chunk((0, 0, 0, 512), psSe, psVe, psDe)

            # ---- pass 2: causal attention in scoresT layout ----
            for b in range(B):
                with (
                    tc.tile_pool(name=f"psS{b}", bufs=4, space="PSUM") as psS,
                    tc.tile_pool(name=f"psV{b}", bufs=1, space="PSUM") as psV,
                    tc.tile_pool(name=f"psD{b}", bufs=1, space="PSUM") as psD,
                ):
                    if b == 0:
                        members = [(0, 1, 0, 512), (0, 2, 0, 512),
                                   (0, 3, 0, 512)]
                    else:
                        members = [(1, 0, 0, 512), (1, 1, 0, 512),
                                   (1, 2, 0, 512), (1, 3, 0, 512)]
                    for m in members:             # query chunks
                        attn_chunk(m, psS, psV, psD)
                        # one deferred b1 rope window per b0 chunk, emitted
                        # right after the finalize chain on the DVE queue
                        if b == 0 and rope_deferred:
                            halves_d, cos_d, sin_d, lw_d = rope_deferred.pop(0)
                            for h in range(2):
                                rope_unit(halves_d[h][0][:], halves_d[h][1][:],
                                          cos_d, sin_d,
                                          qrot[1][0:64, h, lw_d],
                                          qrot[1][64:128, h, lw_d])
                            rope_unit(halves_d[2][0][:], halves_d[2][1][:],
                                      cos_d, sin_d,
                                      krot[1][0:64, lw_d], krot[1][64:128, lw_d])
                    # all wo blocks drain after the last exchange fires: the
                    # ~56us of PE work here fully covers the CC stream tail
                    if b == B - 1:
                        while wo_ready:
                            wo_block(wo_ready.pop(0), psS)

    nc.compile()
    return nc


def _prep_inputs(x, freqs_cos, freqs_sin, wq, wk, wv, wo):
    x = np.asarray(x, np.float32).reshape(NS, DIM)
    xT = np.ascontiguousarray(
        x.T.reshape(MB, P, 8, 512).transpose(0, 2, 1, 3)).astype(BF)
    cos = np.asarray(freqs_cos, np.float32)
    sin = np.asarray(freqs_sin, np.float32)
    cosT = np.ascontiguousarray(np.tile(cos, (B, 1)).T).astype(BF)
    sinT = np.ascontiguousarray(np.tile(sin, (B, 1)).T).astype(BF)

    perm = np.r_[np.arange(0, HD, 2), np.arange(1, HD, 2)]
    scale = np.float32(1.0 / np.sqrt(HD))
    wq = np.asarray(wq, np.float32) * scale
    wk = np.asarray(wk, np.float32)
    wv = np.asarray(wv, np.float32)
    wo = np.asarray(wo, np.float32)

    tri = np.triu(np.ones((P, P), np.float32)).astype(BF)

    def tile_w(w):
        # (2048, d) -> (128, 16*d): row mi holds [mb, d] contiguously
        d = w.shape[1]
        return np.ascontiguousarray(
            w.reshape(MB, P, d).transpose(1, 0, 2).reshape(P, MB * d)).astype(BF)

    in_maps = []
    for c in range(NC):
        wq_c = wq[:, c * 256:(c + 1) * 256]
        wq_cp = np.concatenate([wq_c[:, h * HD + perm] for h in range(2)], axis=1)
        in_maps.append({
            "xT": xT,
            "wq_c": tile_w(wq_cp),
            "wk_c": tile_w(wk[:, c * HD:(c + 1) * HD][:, perm]),
            "wv_c": tile_w(wv[:, c * HD:(c + 1) * HD]),
            "wo_c": tile_w(wo[:, c * 256:(c + 1) * 256]),
            "cosT": cosT,
            "sinT": sinT,
            "tri": tri,
        })
    return in_maps


def _run(inputs, trace=False, **kw):
    from concourse.bass_utils import run_bass_kernel_spmd

    if "nc" not in _cache:
        _cache["nc"] = _build()
    nc = _cache["nc"]
    in_maps = _prep_inputs(**inputs)
    res = run_bass_kernel_spmd(
        nc, in_maps, core_ids=list(range(NC)), trace=trace, **kw
    )
    out = np.empty((NS, DIM), np.float32)
    for c in range(NC):
        out[:, c * 256:(c + 1) * 256] = np.asarray(
            res.results[c]["outT"], dtype=np.float32).T
    return out.reshape(B, S, DIM), res


def kernel(**inputs) -> np.ndarray:
    out, _ = _run(inputs, trace=False)
    return out


# revision 68
# speedup vs baseline: 1.1221x; 1.0268x over previous
"""Trainium2 Bass kernel for GQA attention (B=2, S=2048, DIM=2048, H=16, KV=8,
HD=128) with RoPE + causal mask + output projection.

Sharding: 8-way tensor parallelism over heads. Core c computes q heads
{2c, 2c+1} and kv head c end-to-end (QKV projection, RoPE, causal attention),
contributes its transposed attention output to on-device AllGathers (Shared
output buffers), then computes the output-projection column slice
out[:, 256c:256(c+1)] from the gathered activations. The host only slices
inputs and concatenates outputs.

v5 pipeline notes:
- softmax denominator accumulated on the PE (ones-vector matmul per j block,
  PSUM-accumulated alongside the AV matmul) instead of a DVE add chain.
- projection PSUM windows evicted to bf16 SBUF immediately (ACT for the even
  halves, DVE PSUM-read-port copies for the odd halves); RoPE runs
  SBUF->SBUF in bf16 on the DVE off the PE critical path.
- finalize per chunk: reciprocal_approx_fast on the [1,512] denominator,
  ones-row broadcast matmul, ACT copy to bf16, one DVE multiply per head.
- wo block for chunk (b,t) is emitted two attention chunks after its
  AllGather fires, so the PE arrives after the collective completes; gather
  loads ride the gpsimd queue so they can never head-of-line-block the xt
  stream on the sync queue.
- AllGather outputs are Shared-address-space DRAM (faster RDH path).

Layout tricks:
- everything computed transposed (feature dim on SBUF partitions); only
  on-device transposes are 16 PE transposes per batch for v.
- RoPE interleaved pairs handled by permuting wq/wk columns on the host to
  [evens, odds] per head; q/k permuted consistently so dot products are
  unchanged; v / wo stay unpermuted.
- softmax in scoresT layout (keys on partitions): no max subtraction (scores
  are O(5)), causal mask as a -30 additive bias accumulated by the PE
  (identity matmul) on diagonal blocks only.
- matmuls bf16 (fp32 accumulate); 1/sqrt(HD) folded into wq.
"""

import sys

if "/opt/trn_rl_repo" not in sys.path:
    sys.path.insert(0, "/opt/trn_rl_repo")

import numpy as np
import ml_dtypes

B, S, DIM = 2, 2048, 2048
H, KV, HD = 16, 8, 128
NC = 8
NS = B * S            # 4096 flattened (b, s) rows
P = 128
MB = DIM // P         # 16 contraction blocks for the projections
BF = ml_dtypes.bfloat16

_cache: dict = {}


def _build(debug=False):
    import concourse.bass as bass
    import concourse.mybir as mybir
    import concourse.tile as tile
    from concourse import bacc
    from concourse.masks import make_identity

    dt = mybir.dt
    f32, bf16 = dt.float32, dt.bfloat16
    Exp = mybir.ActivationFunctionType.Exp

    nc = bacc.Bacc("TRN2", debug=False, target_bir_lowering=False, num_devices=NC)

    # x^T arrives pre-tiled as [m_block, window, 128, 512] so every
    # projection-stream DMA is one contiguous 128KB block
    xT_h = nc.dram_tensor("xT", (MB, 8, P, 512), bf16, kind="ExternalInput").ap()
    # weights arrive pre-tiled as [mi=128, mb*d] so their DMAs are contiguous
    # wq m-block 0 arrives as its own small tensor so the first matmul can
    # start ~3us earlier
    wqa_h = nc.dram_tensor("wqa_c", (P, 256), bf16, kind="ExternalInput").ap()
    wq_h = nc.dram_tensor("wq_c", (P, (MB - 1) * 256), bf16,
                          kind="ExternalInput").ap()
    wk_h = nc.dram_tensor("wk_c", (P, MB * HD), bf16, kind="ExternalInput").ap()
    wv_h = nc.dram_tensor("wv_c", (P, MB * HD), bf16, kind="ExternalInput").ap()
    wo_h = nc.dram_tensor("wo_c", (P, MB * 256), bf16, kind="ExternalInput").ap()
    cos_h = nc.dram_tensor("cosT", (64, NS), bf16, kind="ExternalInput").ap()
    sin_h = nc.dram_tensor("sinT", (64, NS), bf16, kind="ExternalInput").ap()
    # 0/1 upper-triangle for the within-block causal mask
    tri_h = nc.dram_tensor("tri", (P, P), bf16, kind="ExternalInput").ap()
    out_h = nc.dram_tensor("outT", (256, NS), bf16, kind="ExternalOutput").ap()

    with tile.TileContext(nc) as tc:
        with (
            tc.tile_pool(name="const", bufs=1) as const,
            tc.tile_pool(name="persist", bufs=1) as persist,
            tc.tile_pool(name="xs", bufs=16) as xs,
            tc.tile_pool(name="tmp", bufs=3) as tmp,
            tc.tile_pool(name="qk", bufs=2) as qk,
            tc.tile_pool(name="qk1", bufs=1) as qk1,
            tc.tile_pool(name="et", bufs=10) as et,
            tc.tile_pool(name="gp", bufs=8) as gp,
            tc.tile_pool(name="ot", bufs=3) as ot,
            tc.tile_pool(name="dram", bufs=1, space="DRAM") as dram,
        ):
            # ---- constants into SBUF ----
            # wq on the sync queue ahead of the xt stream; everything else on
            # the gpsimd queue.
            wq_sb = const.tile([P, MB, 256], bf16)
            nc.sync.dma_start(wq_sb[:, 0, :], wqa_h)
            nc.sync.dma_start(
                wq_sb[:, 1:MB, :],
                wq_h.rearrange("p (mb d) -> p mb d", mb=MB - 1))
            wk_sb = const.tile([P, MB, HD], bf16)
            nc.gpsimd.dma_start(wk_sb[:], wk_h.rearrange("p (mb d) -> p mb d", mb=MB))
            wv_sb = const.tile([P, MB, HD], bf16)
            nc.gpsimd.dma_start(wv_sb[:], wv_h.rearrange("p (mb d) -> p mb d", mb=MB))
            cos_sb = const.tile([64, NS], bf16)
            nc.gpsimd.dma_start(cos_sb[:], cos_h)
            sin_sb = const.tile([64, NS], bf16)
            nc.gpsimd.dma_start(sin_sb[:], sin_h)
            wo_sb = const.tile([P, MB, 256], bf16)
            nc.gpsimd.dma_start(wo_sb[:], wo_h.rearrange("p (mb d) -> p mb d", mb=MB))
            tri_sb = const.tile([P, P], bf16)
            nc.gpsimd.dma_start(tri_sb[:], tri_h)
            ones_sb = const.tile([P, 1], bf16)
            nc.gpsimd.memset(ones_sb[:], 1.0)
            ident = const.tile([P, P], bf16)
            make_identity(nc, ident[:])

            # ---- per-batch persistent activations ----
            qrot = [persist.tile([P, 2, S], bf16, name=f"qrot{b}") for b in range(B)]
            krot = [persist.tile([P, S], bf16, name=f"krot{b}") for b in range(B)]
            vTt = [persist.tile([P, S], bf16, name=f"vTt{b}") for b in range(B)]
            vnat = [persist.tile([P, S // P, HD], bf16, name=f"vnat{b}")
                    for b in range(B)]
            oav = [persist.tile([P, 2, S], bf16, name=f"oav{b}") for b in range(B)]
            # exchange groups: chunks sharing one AllGather (batching
            # amortizes the ~9us fixed collective cost). Members are
            # (b, t, qlo, qhi) in processing order; the final chunk is split
            # into two 256-column halves so the tail collective is small.
            GROUPS = [
                [(0, 0, 0, 512)],
                [(0, 1, 0, 512), (0, 2, 0, 512)],
                [(0, 3, 0, 512), (1, 0, 0, 512)],
                [(1, 1, 0, 512), (1, 2, 0, 512)],
                [(1, 3, 0, 512)],
            ]
            slot_of = {}
            for gi, members in enumerate(GROUPS):
                for si, m in enumerate(members):
                    slot_of[m] = (gi, si)
            ag_in = [dram.tile([len(m) * 256, m[0][3] - m[0][2]], bf16,
                               name=f"agi{gi}")
                     for gi, m in enumerate(GROUPS)]
            ag_out = [dram.tile([NC * len(m) * 256, m[0][3] - m[0][2]], bf16,
                                name=f"ago{gi}", addr_space="Shared")
                      for gi, m in enumerate(GROUPS)]

            # tiny warmup AllGather fired during the projection phase so the
            # first real collective doesn't pay the slow-start cost
            warm_in = dram.tile([1, 512], bf16, name="warm_in")
            warm_out = dram.tile([NC, 512], bf16, name="warm_out",
                                 addr_space="Shared")
            nc.gpsimd.dma_start(warm_in[:], cos_h[0:1, 0:512])
            nc.gpsimd.collective_compute(
                "AllGather",
                mybir.AluOpType.bypass,
                replica_groups=[list(range(NC))],
                ins=[warm_in.opt()],
                outs=[warm_out.opt()],
            )

            def rope_unit(xe, xo, cos_c, sin_c, out_even, out_odd):
                # xe/xo are bf16 base-0 SBUF copies of the window's two halves
                t1 = tmp.tile([64, 512], bf16, tag="r1", name="r1")
                t2 = tmp.tile([64, 512], bf16, tag="r2", name="r2")
                nc.vector.tensor_mul(t1[:], xe, cos_c)
                nc.vector.tensor_mul(t2[:], xo, sin_c)
                nc.vector.tensor_sub(out_even, t1[:], t2[:])
                t3 = tmp.tile([64, 512], bf16, tag="r1", name="r3")
                t4 = tmp.tile([64, 512], bf16, tag="r2", name="r4")
                nc.vector.tensor_mul(t3[:], xe, sin_c)
                nc.vector.tensor_mul(t4[:], xo, cos_c)
                nc.vector.tensor_add(out_odd, t3[:], t4[:])

            def wo_exchange(m):
                """Stage member m=(b,t,qlo,qhi)'s attention output into its
                group slot; fire the group's AllGather once its last member
                is staged."""
                b, t, qlo, qhi = m
                gi, si = slot_of[m]
                il = slice(t * 512 + qlo, t * 512 + qhi)
                for h in range(2):
                    nc.gpsimd.dma_start(
                        ag_in[gi][si * 256 + h * P: si * 256 + (h + 1) * P, :],
                        oav[b][:, h, il])
                if si == len(GROUPS[gi]) - 1:
                    nc.gpsimd.collective_compute(
                        "AllGather",
                        mybir.AluOpType.bypass,
                        replica_groups=[list(range(NC))],
                        ins=[ag_in[gi].opt()],
                        outs=[ag_out[gi].opt()],
                    )

            def wo_block(m, pool):
                """Output-projection column slice for member m: gathered rows
                arrive as paired loads alternating between the gpsimd and
                sync queues, then 32 PE matmuls + bf16 store."""
                b, t, qlo, qhi = m
                w = qhi - qlo
                gi, si = slot_of[m]
                rs = len(GROUPS[gi]) * 256   # gathered rows per core
                pw = [pool.tile([P, 512], f32, tag="ps", name=f"pw{n}")
                      for n in range(2)]
                for rp in range(MB // 2):
                    g2 = gp.tile([P, 2, 512], bf16, tag="g", name="g")
                    base = rp * rs + si * 256
                    src = ag_out[gi][base:base + 256, :].rearrange(
                        "(two p) q -> p two q", two=2)
                    if rp % 2 == 0:
                        nc.gpsimd.dma_start(g2[:, :, 0:w], src)
                    else:
                        nc.sync.dma_start(g2[:, :, 0:w], src)
                    for half in range(2):
                        r = rp * 2 + half
                        for n in range(2):
                            nc.tensor.matmul(
                                pw[n][:, 0:w],
                                wo_sb[:, r, n * 128:(n + 1) * 128],
                                g2[:, half, 0:w],
                                start=(r == 0), stop=(r == MB - 1),
                            )
                for n in range(2):
                    o = ot.tile([P, 512], bf16, tag="o", name="o")
                    nc.scalar.copy(o[:, 0:w], pw[n][:, 0:w])
                    nc.sync.dma_start(
                        out_h[n * P:(n + 1) * P,
                              b * S + t * 512 + qlo: b * S + t * 512 + qhi],
                        o[:, 0:w],
                    )

            # wo blocks become runnable when their group's AllGather fires
            wo_ready = []
            # b1 rope units deferred into the b0 attention phase
            rope_deferred = []

            def attn_chunk(m, psS, psV, psD):
                """Causal attention for query cols [qlo,qhi) of chunk (b,t)
                in scoresT layout, incl. softmax finalize and exchange."""
                b, t, qlo, qhi = m
                w = qhi - qlo
                il = slice(t * 512 + qlo, t * 512 + qhi)
                pav = [psV.tile([P, 512], f32, tag=f"pav{h}",
                                name=f"pav{h}") for h in range(2)]
                pden = [psD.tile([1, 512], f32, tag=f"pden{h}",
                                 name=f"pden{h}") for h in range(2)]
                nj = (t * 512 + qhi - 1) // P + 1
                # descending j: masked diagonal blocks run first so the
                # drain at the end only waits on plain exps
                order = list(range(nj - 1, -1, -1))

                def av_den(e, idx, j, h):
                    nc.tensor.matmul(
                        pden[h][:, 0:w], ones_sb[:], e[:, 0:w],
                        start=(idx == 0), stop=(idx == nj - 1),
                    )
                    nc.tensor.matmul(
                        pav[h][:, 0:w], vnat[b][:, j, :], e[:, 0:w],
                        start=(idx == 0), stop=(idx == nj - 1),
                    )

                pipe = []
                for idx, j in enumerate(order):
                    for h in range(2):
                        # column offset of key block j's diagonal in this
                        # query window
                        moff = j * P - (t * 512 + qlo)
                        ps = psS.tile([P, 512], f32, tag="ps", name="ps")
                        nc.tensor.matmul(
                            ps[:, 0:w], krot[b][:, j * P:(j + 1) * P],
                            qrot[b][:, h, il], start=True, stop=True,
                        )
                        e = et.tile([P, 512], bf16, tag="e", name="e")
                        if moff > 0:
                            # diagonal block: exp only the live columns;
                            # DVE zeroes the masked ones
                            nc.scalar.activation(
                                e[:, moff:w], ps[:, moff:w], Exp)
                            nc.vector.memset(e[:, 0:moff], 0.0)
                        else:
                            nc.scalar.activation(e[:, 0:w], ps[:, 0:w], Exp)
                        if moff >= 0:
                            # 0/1 triangle multiply on the diagonal
                            nc.vector.tensor_mul(
                                e[:, moff:moff + P],
                                e[:, moff:moff + P], tri_sb[:])
                        pipe.append((e, idx, j, h))
                    while len(pipe) > 6:
                        av_den(*pipe.pop(0))
                for item in pipe:
                    av_den(*item)

                # finalize: rcp(den), gpsimd partition broadcast, one DVE
                # multiply per head
                for h in range(2):
                    rcp = tmp.tile([1, 512], f32, tag="rcp", name="rcp")
                    nc.vector.reciprocal_approx_fast(rcp[:, 0:w],
                                                     pden[h][:, 0:w])
                    rcp_bf = tmp.tile([1, 512], bf16, tag="rcpc", name="rcpc")
                    nc.vector.tensor_copy(rcp_bf[:, 0:w], rcp[:, 0:w])
                    rcp_b = tmp.tile([P, 512], bf16, tag="rcpb", name="rcpb")
                    nc.gpsimd.partition_broadcast(rcp_b[:, 0:w],
                                                  rcp_bf[:, 0:w])
                    nc.vector.tensor_mul(oav[b][:, h, il],
                                         pav[h][:, 0:w], rcp_b[:, 0:w])

                wo_exchange(m)
                gi, si = slot_of[m]
                if si == len(GROUPS[gi]) - 1:
                    wo_ready.extend(GROUPS[gi])

            # ---- pass 1: projections + transposes for both batches, with
            # b0's first attention chunk squeezed between them so the
            # AllGather stream starts ~100us earlier ----
            for b in range(B):
                with tc.tile_pool(name=f"psA{b}", bufs=2, space="PSUM") as psA:
                    for sp in range(4):          # 512-col windows within batch
                        gw = slice(b * S + sp * 512, b * S + (sp + 1) * 512)
                        lw = slice(sp * 512, (sp + 1) * 512)
                        pq = [psA.tile([P, 512], f32, tag=f"pq{h}", name=f"pq{h}")
                              for h in range(2)]
                        pk = psA.tile([P, 512], f32, tag="pk", name="pk")
                        pv = psA.tile([P, 512], f32, tag="pv", name="pv")
                        for m in range(MB):
                            xt = xs.tile([P, 512], bf16, tag="xt", name="xt")
                            nc.sync.dma_start(xt[:], xT_h[m, b * 4 + sp])
                            for acc, lhsT in (
                                (pq[0], wq_sb[:, m, 0:128]),
                                (pq[1], wq_sb[:, m, 128:256]),
                                (pk, wk_sb[:, m, :]),
                                (pv, wv_sb[:, m, :]),
                            ):
                                nc.tensor.matmul(
                                    acc[:], lhsT, xt[:],
                                    start=(m == 0), stop=(m == MB - 1),
                                )
                        # evict the PSUM banks to bf16 SBUF right away: even
                        # halves via ACT, odd halves via the DVE PSUM read
                        # port (which supports the base-64 partition offset).
                        # b1's last two windows get dedicated tiles because
                        # their RoPE is deferred into the b0 attention phase
                        # (keeps the DVE queue clear for b0's finalize chains)
                        defer = b == 1 and sp >= 2
                        halves = []
                        for u, src in enumerate((pq[0], pq[1], pk)):
                            if defer:
                                xe = qk1.tile([64, 512], bf16,
                                              tag=f"dxe{sp}{u}", name=f"xe{u}")
                                xo = qk1.tile([64, 512], bf16,
                                              tag=f"dxo{sp}{u}", name=f"xo{u}")
                            else:
                                xe = qk.tile([64, 512], bf16, tag=f"xe{u}",
                                             name=f"xe{u}")
                                xo = qk.tile([64, 512], bf16, tag=f"xo{u}",
                                             name=f"xo{u}")
                            nc.scalar.copy(xe[:], src[0:64, :])
                            nc.vector.tensor_copy(xo[:], src[64:128, :])
                            halves.append((xe, xo))
                        nc.scalar.copy(vTt[b][:, lw], pv[:])
                        # RoPE runs SBUF->SBUF on the DVE, off the PE path
                        cos_c, sin_c = cos_sb[:, gw], sin_sb[:, gw]
                        if defer:
                            rope_deferred.append((halves, cos_c, sin_c, lw))
                        else:
                            for h in range(2):
                                rope_unit(halves[h][0][:], halves[h][1][:],
                                          cos_c, sin_c,
                                          qrot[b][0:64, h, lw],
                                          qrot[b][64:128, h, lw])
                            rope_unit(halves[2][0][:], halves[2][1][:],
                                      cos_c, sin_c,
                                      krot[b][0:64, lw], krot[b][64:128, lw])

                # ---- v natural layout via PE transposes ----
                with tc.tile_pool(name=f"psT{b}", bufs=2, space="PSUM") as psT:
                    for blk in range(S // P):
                        pt = psT.tile([P, P], bf16, tag="pt", name="pt")
                        nc.tensor.transpose(
                            pt[:], vTt[b][:, blk * P:(blk + 1) * P], ident[:])
                        nc.scalar.copy(vnat[b][:, blk, :], pt[:])

                if b == 0:
                    # early attention chunk (0,0): fires the first real
                    # AllGather while b1's projections still run
                    with (
                        tc.tile_pool(name="psSe", bufs=3, space="PSUM") as psSe,
                        tc.tile_pool(name="psVe", bufs=1, space="PSUM") as psVe,
                        tc.tile_pool(name="psDe", bufs=1, space="PSUM") as psDe,
                    ):
                        attn_chunk((0, 0, 0, 512), psSe, psVe, psDe)

            # ---- pass 2: causal attention in scoresT layout ----
            for b in range(B):
                with (
                    tc.tile_pool(name=f"psS{b}", bufs=4, space="PSUM") as psS,
                    tc.tile_pool(name=f"psV{b}", bufs=1, space="PSUM") as psV,
                    tc.tile_pool(name=f"psD{b}", bufs=1, space="PSUM") as psD,
                ):
                    if b == 0:
                        members = [(0, 1, 0, 512), (0, 2, 0, 512),
                                   (0, 3, 0, 512)]
                    else:
                        members = [(1, 0, 0, 512), (1, 1, 0, 512),
                                   (1, 2, 0, 512), (1, 3, 0, 512)]
                    for m in members:             # query chunks
                        attn_chunk(m, psS, psV, psD)
                        # one deferred b1 rope window per b0 chunk, emitted
                        # right after the finalize chain on the DVE queue
                        if b == 0 and rope_deferred:
                            halves_d, cos_d, sin_d, lw_d = rope_deferred.pop(0)
                            for h in range(2):
                                rope_unit(halves_d[h][0][:], halves_d[h][1][:],
                                          cos_d, sin_d,
                                          qrot[1][0:64, h, lw_d],
                                          qrot[1][64:128, h, lw_d])
                            rope_unit(halves_d[2][0][:], halves_d[2][1][:],
                                      cos_d, sin_d,
                                      krot[1][0:64, lw_d], krot[1][64:128, lw_d])
                    # all wo blocks drain after the last exchange fires: the
                    # ~56us of PE work here fully covers the CC stream tail
                    if b == B - 1:
                        while wo_ready:
                            wo_block(wo_ready.pop(0), psS)

    nc.compile()
    return nc


def _prep_inputs(x, freqs_cos, freqs_sin, wq, wk, wv, wo):
    x = np.asarray(x, np.float32).reshape(NS, DIM)
    xT = np.ascontiguousarray(
        x.T.reshape(MB, P, 8, 512).transpose(0, 2, 1, 3)).astype(BF)
    cos = np.asarray(freqs_cos, np.float32)
    sin = np.asarray(freqs_sin, np.float32)
    cosT = np.ascontiguousarray(np.tile(cos, (B, 1)).T).astype(BF)
    sinT = np.ascontiguousarray(np.tile(sin, (B, 1)).T).astype(BF)

    perm = np.r_[np.arange(0, HD, 2), np.arange(1, HD, 2)]
    scale = np.float32(1.0 / np.sqrt(HD))
    wq = np.asarray(wq, np.float32) * scale
    wk = np.asarray(wk, np.float32)
    wv = np.asarray(wv, np.float32)
    wo = np.asarray(wo, np.float32)

    tri = np.triu(np.ones((P, P), np.float32)).astype(BF)

    def tile_w(w):
        # (2048, d) -> (128, 16*d): row mi holds [mb, d] contiguously
        d = w.shape[1]
        return np.ascontiguousarray(
            w.reshape(MB, P, d).transpose(1, 0, 2).reshape(P, MB * d)).astype(BF)

    in_maps = []
    for c in range(NC):
        wq_c = wq[:, c * 256:(c + 1) * 256]
        wq_cp = np.concatenate([wq_c[:, h * HD + perm] for h in range(2)], axis=1)
        wq_t = tile_w(wq_cp)
        in_maps.append({
            "xT": xT,
            "wqa_c": np.ascontiguousarray(wq_t[:, 0:256]),
            "wq_c": np.ascontiguousarray(wq_t[:, 256:]),
            "wk_c": tile_w(wk[:, c * HD:(c + 1) * HD][:, perm]),
            "wv_c": tile_w(wv[:, c * HD:(c + 1) * HD]),
            "wo_c": tile_w(wo[:, c * 256:(c + 1) * 256]),
            "cosT": cosT,
            "sinT": sinT,
            "tri": tri,
        })
    return in_maps


def _run(inputs, trace=False, **kw):
    from concourse.bass_utils import run_bass_kernel_spmd

    if "nc" not in _cache:
        _cache["nc"] = _build()
    nc = _cache["nc"]
    in_maps = _prep_inputs(**inputs)
    res = run_bass_kernel_spmd(
        nc, in_maps, core_ids=list(range(NC)), trace=trace, **kw
    )
    out = np.empty((NS, DIM), np.float32)
    for c in range(NC):
        out[:, c * 256:(c + 1) * 256] = np.asarray(
            res.results[c]["outT"], dtype=np.float32).T
    return out.reshape(B, S, DIM), res


def kernel(**inputs) -> np.ndarray:
    out, _ = _run(inputs, trace=False)
    return out
